# revision 7
# baseline (speedup 1.0000x reference)
"""Causal linear attention (elu+1 feature map) on 8 Trainium2 NeuronCores.

Full inputs (n=2, l=2048, h=8, d=64) fp32 are sharded over the 16 (n,h)
head-sequences: core i handles pairs (2i, 2i+1). Each core runs a chunked
scan (chunk=128) over the sequence:

  out[i] = (ATm.T @ Vaug + QfT.T @ Saug) * 1/denom
  Saug  += Kf.T @ Vaug            (PSUM-resident running state, fp32)

with the feature map elu(x)+1 computed as min(exp(x), x+1 clamped at 1)
 == exp(x) for x<=0, x+1 for x>0 (exp(x) >= x+1 everywhere).

Host-side: inputs are cast to fp16 and laid out as (128, 2048) per core:
[i, 128*c + 64*p + d] so every DMA is contiguous. Matmuls run in fp16
(1 cyc/row on PE vs 4 for fp32); PSUM accumulation and the output stay fp32.

PSUM accumulation banks are zero-initialized by a K=1 all-zeros matmul that
owns the bank's only start=True; all real matmuls accumulate (start=False),
making them order-robust (PSUM start=True invalidates a whole 2KB bank).
"""
import numpy as np
from contextlib import ExitStack

import concourse.bacc as bacc
import concourse.tile as tile
from concourse import mybir
from concourse.bass_utils import run_bass_kernel_spmd
from concourse.masks import make_identity

N, L, H, D = 2, 2048, 8, 64
C = 128                 # chunk (seq positions per chunk)
NCH = L // C            # 16 chunks
GROUP = 4               # chunks per fmap/DMA group
NGRP = NCH // GROUP
PAIRS = 2               # (n,h) pairs per core
W = NCH * PAIRS * D     # 2048 free columns per tensor per core
GW = GROUP * PAIRS * D  # 512 free columns per group
VW = GROUP * PAIRS * (D + 1)  # v group tile width (ones column appended)

f16 = mybir.dt.float16
f32 = mybir.dt.float32
AF = mybir.ActivationFunctionType
OP = mybir.AluOpType


def build_kernel():
    nc = bacc.Bacc("TRN2", target_bir_lowering=False, debug=False, num_devices=8)
    q_d = nc.dram_tensor("q", (C, W), f16, kind="ExternalInput").ap()
    k_d = nc.dram_tensor("k", (C, W), f16, kind="ExternalInput").ap()
    v_d = nc.dram_tensor("v", (C, W), f16, kind="ExternalInput").ap()
    o_d = nc.dram_tensor("o", (C, W), f32, kind="ExternalOutput").ap()

    with tile.TileContext(nc) as tc, ExitStack() as ctx:
        consts = ctx.enter_context(tc.tile_pool(name="consts", bufs=1))
        io_pool = ctx.enter_context(tc.tile_pool(name="io", bufs=2))
        fm_pool = ctx.enter_context(tc.tile_pool(name="fm", bufs=2))
        sm_pool = ctx.enter_context(tc.tile_pool(name="sm", bufs=3))
        tp_psum = ctx.enter_context(tc.tile_pool(name="tp", bufs=2, space="PSUM"))
        at_psum = ctx.enter_context(tc.tile_pool(name="at", bufs=2, space="PSUM"))
        out_psum = ctx.enter_context(tc.tile_pool(name="out", bufs=3, space="PSUM"))
        s_psum = ctx.enter_context(tc.tile_pool(name="sp", bufs=1, space="PSUM"))

        ident = consts.tile([C, C], f16)
        make_identity(nc, ident)

        zeros = consts.tile([1, C + PAIRS], f16)
        nc.vector.memset(zeros, 0.0)

        # maskT[j, i] = 1 where j <= i (upper triangular incl diag), both pair blocks
        maskT = consts.tile([C, PAIRS * C], f32)
        m3 = maskT.rearrange("j (b i) -> j b i", b=PAIRS)
        nc.gpsimd.memset(maskT, 0.0)
        nc.gpsimd.affine_select(
            out=m3, in_=m3, compare_op=OP.is_gt, fill=1.0,
            base=0, pattern=[[0, PAIRS], [-1, C]], channel_multiplier=1,
        )

        # persistent running state: per pair p cols [65p, 65p+64]:
        # cols 65p..65p+63 = S (d x m), col 65p+64 = ksum
        S_ps = s_psum.tile([D, PAIRS * (D + 1)], f32)
        nc.tensor.matmul(S_ps, zeros[:, 0:D], zeros[:, 0:PAIRS * (D + 1)],
                         start=True, stop=False, skip_group_check=True)

        S_sb = None
        for g in range(NGRP):
            # ---- group loads ----
            q_g = io_pool.tile([C, GW], f16, tag="q_g")
            k_g = io_pool.tile([C, GW], f16, tag="k_g")
            v_g = io_pool.tile([C, VW], f16, tag="v_g")
            cols = slice(g * GW, (g + 1) * GW)
            nc.sync.dma_start(q_g, q_d[:, cols])
            nc.sync.dma_start(k_g, k_d[:, cols])
            v4 = v_g.rearrange("i (j b x) -> i j b x", j=GROUP, b=PAIRS)
            nc.sync.dma_start(
                v4[:, :, :, 0:D],
                v_d[:, cols].rearrange("i (j b x) -> i j b x", j=GROUP, b=PAIRS),
            )
            nc.vector.memset(v4[:, :, :, D:D + 1], 1.0)

            # ---- feature maps in natural layout: f = min(exp(x), max(x+1,1)) ----
            e_q = fm_pool.tile([C, GW], f16, tag="e_q")
            r_q = fm_pool.tile([C, GW], f16, tag="r_q")
            qf_g = fm_pool.tile([C, GW], f16, tag="qf")
            nc.scalar.activation(e_q, q_g, AF.Exp)
            nc.vector.tensor_scalar_max(r_q, q_g, 0.0)
            nc.vector.scalar_tensor_tensor(
                out=qf_g, in0=e_q, scalar=1.0, in1=r_q, op0=OP.min, op1=OP.add)

            e_k = fm_pool.tile([C, GW], f16, tag="e_k")
            r_k = fm_pool.tile([C, GW], f16, tag="r_k")
            kf_g = fm_pool.tile([C, GW], f16, tag="kf")
            nc.scalar.activation(e_k, k_g, AF.Exp)
            nc.vector.tensor_scalar_max(r_k, k_g, 0.0)
            nc.vector.scalar_tensor_tensor(
                out=kf_g, in0=e_k, scalar=1.0, in1=r_k, op0=OP.min, op1=OP.add)

            # ---- pair-split transposes of the feature-mapped tensors ----
            # tq_ps[:, (2j+p)*128 + i] = Qf[i, 128j + 64p + d] transposed -> (d, i)
            tq_ps = tp_psum.tile([D, GROUP * PAIRS * C], f16, tag="tp")
            for j in range(GROUP):
                for p in range(PAIRS):
                    nc.tensor.transpose(
                        tq_ps[:, (2 * j + p) * C:(2 * j + p + 1) * C],
                        qf_g[:, j * PAIRS * D + p * D:j * PAIRS * D + (p + 1) * D],
                        ident)
            qfT = fm_pool.tile([D, GROUP * PAIRS * C], f16, tag="qfT")
            nc.scalar.copy(qfT, tq_ps)

            tk_ps = tp_psum.tile([D, GROUP * PAIRS * C], f16, tag="tp")
            for j in range(GROUP):
                for p in range(PAIRS):
                    nc.tensor.transpose(
                        tk_ps[:, (2 * j + p) * C:(2 * j + p + 1) * C],
                        kf_g[:, j * PAIRS * D + p * D:j * PAIRS * D + (p + 1) * D],
                        ident)
            kfT = fm_pool.tile([D, GROUP * PAIRS * C], f16, tag="kfT")
            nc.vector.tensor_copy(kfT, tk_ps)

            stage = io_pool.tile([C, GW], f32, tag="stage")

            # ---- per-chunk scan ----
            for j in range(GROUP):
                c = g * GROUP + j
                # AT[jj, i] = Kf @ QfT per pair
                at_ps = at_psum.tile([C, PAIRS * C], f32, tag="at")
                for p in range(PAIRS):
                    tcol = slice((2 * j + p) * C, (2 * j + p + 1) * C)
                    nc.tensor.matmul(
                        at_ps[:, p * C:(p + 1) * C],
                        kfT[:, tcol], qfT[:, tcol],
                        start=True, stop=True)

                atm = sm_pool.tile([C, PAIRS * C], f16, tag="atm")
                nc.vector.tensor_mul(atm, at_ps, maskT)

                # state snapshot for this chunk (state after chunk c-1)
                if c > 0:
                    S_sb = sm_pool.tile([D, PAIRS * (D + 1)], f16, tag="s_sb")
                    nc.scalar.copy(S_sb, S_ps)

                # out_ps: [out | denom] per pair at cols [65p, 65p+65)
                out_ps = out_psum.tile([C, PAIRS * (D + 1)], f32, tag="out")
                nc.tensor.matmul(out_ps, zeros[:, 0:C],
                                 zeros[:, 0:PAIRS * (D + 1)],
                                 start=True, stop=False, skip_group_check=True)
                for p in range(PAIRS):
                    vs = slice(p * (D + 1), (p + 1) * (D + 1))
                    tcol = slice((2 * j + p) * C, (2 * j + p + 1) * C)
                    vaug = v4[:, j, p, :]
                    nc.tensor.matmul(
                        out_ps[:, vs], atm[:, p * C:(p + 1) * C], vaug,
                        start=False, stop=(c == 0 and p == PAIRS - 1),
                        skip_group_check=True)
                    if c > 0:
                        nc.tensor.matmul(
                            out_ps[:, vs],
                            qfT[:, tcol], S_sb[:, vs],
                            start=False, stop=(p == PAIRS - 1),
                            skip_group_check=True)

                    # state update (after snapshot read; Tile orders via WAR)
                    nc.tensor.matmul(
                        S_ps[:, vs],
                        kf_g[:, j * PAIRS * D + p * D:j * PAIRS * D + (p + 1) * D],
                        vaug,
                        start=False, stop=(c == NCH - 1 and p == PAIRS - 1),
                        skip_group_check=True)

                recip = sm_pool.tile([C, PAIRS], f32, tag="recip")
                den = out_ps.rearrange("i (b x) -> i b x", b=PAIRS)[:, :, D]
                nc.vector.reciprocal(recip, den)

                for p in range(PAIRS):
                    nc.scalar.activation(
                        stage[:, j * PAIRS * D + p * D:j * PAIRS * D + (p + 1) * D],
                        out_ps[:, p * (D + 1):p * (D + 1) + D],
                        AF.Copy, scale=recip[:, p:p + 1])

            nc.sync.dma_start(o_d[:, cols], stage)

    nc.compile()
    return nc


_nc_cache = None


def _get_nc():
    global _nc_cache
    if _nc_cache is None:
        _nc_cache = build_kernel()
    return _nc_cache


def _to_core_layout(x, core):
    # (N,L,H,D) fp32 -> (128, 2048) fp16 [i, 128c + 64p + d] for this core's 2 pairs
    flat = x.transpose(0, 2, 1, 3).reshape(N * H, L, D)
    xc = flat[2 * core:2 * core + 2]
    return np.ascontiguousarray(
        xc.reshape(PAIRS, NCH, C, D).transpose(2, 1, 0, 3).reshape(C, W)
    ).astype(np.float16)


def kernel(queries, keys, values):
    nc = _get_nc()
    in_maps = []
    for core in range(8):
        in_maps.append({
            "q": _to_core_layout(queries, core),
            "k": _to_core_layout(keys, core),
            "v": _to_core_layout(values, core),
        })
    res = run_bass_kernel_spmd(nc, in_maps, core_ids=list(range(8)))
    out = np.zeros((N, L, H, D), np.float32)
    for core in range(8):
        oc = res.results[core]["o"].reshape(C, NCH, PAIRS, D)
        oc = oc.transpose(2, 1, 0, 3).reshape(PAIRS, L, D)
        for p in range(PAIRS):
            flat = 2 * core + p
            out[flat // H, :, flat % H, :] = oc[p]
    return out
